# revision 1
# baseline (speedup 1.0000x reference)
"""Trainium2 Bass kernel for nn_Criterion_74809740362369.

Multi-trajectory prediction loss (Laplace NLL + BVG entropy + KL + ADE/FDE
+ scores MSE), data-parallel over the batch dim across 8 NeuronCores.

Math restructuring (validated against the jax reference):
  - Laplace NLL row term: m * (log(2sx) + log(2sy) + |dx|/sx + |dy|/sy)
    with log(2s) computed as ACT ln(scale=2), and 1/s as exp(ln2 - ln(2s)).
  - BVG entropy: sum_t [log(2sx)+log(2sy)] - T*log4 + 0.5*sum_t[ln(1+rho)+ln(1-rho)]
    + T*(1+log(2pi)); the constants fold into one final scalar add.
  - dist = sqrt(dx^2+dy^2) = exp(0.5*ln(dx^2+dy^2))  (avoids the sqrt table set;
    the whole kernel uses the single natural_log_exp ACT table set).
  - KL(post||probs) = -sum_k post*nll - lse, so loss_nll + 20*KL
    = -19*nllpost - 20*lse (no explicit post materialization).
  - final = [sum over all (l,b,n) of main-terms + sum over (b,n,l) of
    (bests-pred)^2] / (B*L*N); each core returns its partial sum as a
    [128,1] per-partition vector, summed on the host.

Each core gets a contiguous batch shard: trajs[:, c*32:(c+1)*32] etc.
"""

from contextlib import ExitStack

import numpy as np

import concourse.bass as bass
import concourse.bacc as bacc
import concourse.tile as tile
from concourse import mybir
from concourse.bass_utils import run_bass_kernel_spmd

F32 = mybir.dt.float32
AF = mybir.ActivationFunctionType
OP = mybir.AluOpType
AX = mybir.AxisListType

L, Bc, N, K, T, C = 4, 32, 16, 6, 80, 5  # per-core shard dims
NCORES = 8
ROWS = L * Bc * N          # 2048 (l,b,n) rows per core
TILES = ROWS // 128        # 16
DROWS = Bc * N             # 512 (b,n) rows per core
DTILES = DROWS // 128      # 4

LN2 = float(np.log(2.0))
C1 = float(1.0 + np.log(2.0 * np.pi))
LOG4 = float(np.log(4.0))
ENT_CONST = 40.0 * T * (C1 - LOG4)  # folded into the final per-row add


def _bcast_outer(ap, n):
    """[P, d...] -> [P, n, d...] with a step-0 (broadcast) dim."""
    return bass.AP(tensor=ap.tensor, offset=ap.offset,
                   ap=[ap.ap[0], [0, n]] + list(ap.ap[1:]))


def _bcast_inner(ap, n):
    """[P, d...] -> [P, d..., n] with a step-0 (broadcast) dim."""
    return bass.AP(tensor=ap.tensor, offset=ap.offset,
                   ap=list(ap.ap) + [[0, n]])


def build_kernel():
    nc = bacc.Bacc("TRN2")
    trajs_d = nc.dram_tensor("trajs", [L, Bc, N, K, T, C], F32, kind="ExternalInput")
    data_d = nc.dram_tensor("data", [Bc, N, T, 3], F32, kind="ExternalInput")
    probs_d = nc.dram_tensor("probs", [L, Bc, N, K], F32, kind="ExternalInput")
    scores_d = nc.dram_tensor("scores", [L, Bc, N, K], F32, kind="ExternalInput")
    out_d = nc.dram_tensor("out", [128, 1], F32, kind="ExternalOutput")

    trajs_r = trajs_d[:].rearrange("l b n k t c -> (l b n) (k t c)")
    data_r = data_d[:].rearrange("b n t c -> (b n) (t c)")
    probs_r = probs_d[:].rearrange("l b n k -> (l b n) k")
    scores_r = scores_d[:].rearrange("l b n k -> (l b n) k")

    with tile.TileContext(nc) as tc, ExitStack() as ctx:
        tp = ctx.enter_context(tc.tile_pool(name="traj", bufs=3))
        dp = ctx.enter_context(tc.tile_pool(name="dat", bufs=1))
        ip = ctx.enter_context(tc.tile_pool(name="inter", bufs=2))
        rp = ctx.enter_context(tc.tile_pool(name="res", bufs=1))

        # ---- packed per-row results, written tile-by-tile, consumed in stage B
        R1 = rp.tile([128, TILES, K], F32)   # sum_t m*w   (nll)
        R2 = rp.tile([128, TILES, K], F32)   # sum_t s     (ent log part)
        R3 = rp.tile([128, TILES, K], F32)   # sum_t va+vb (ent rho part)
        R4 = rp.tile([128, TILES, K], F32)   # sum_t m*dist (ade*T)
        FDE = rp.tile([128, TILES, K], F32)  # dist_{T-1} * m_{T-1}
        P_sb = rp.tile([128, TILES, K], F32)
        S_sb = rp.tile([128, TILES, K], F32)

        # bias constant for exp(ln2 - x) = 2/e^x  (only 0.0/1.0 are builtin)
        bln2 = rp.tile([128, 1], F32)
        nc.vector.memset(bln2, LN2)

        # ---- data tiles (gt/mask), shared across the 4 l-values
        dts = []
        for j in range(DTILES):
            dt_j = dp.tile([128, T * 3], F32, name=f"dt{j}", tag=f"dt{j}")
            nc.sync.dma_start(out=dt_j, in_=data_r[j * 128:(j + 1) * 128, :])
            dts.append(dt_j)

        for i in range(TILES):
            nc.sync.dma_start(out=P_sb[:, i, :], in_=probs_r[i * 128:(i + 1) * 128, :])
            nc.sync.dma_start(out=S_sb[:, i, :], in_=scores_r[i * 128:(i + 1) * 128, :])

        # ================= stage A: heavy per-(k,t) pipeline =================
        for i in range(TILES):
            j = i % DTILES
            dt_j = dts[j]
            dv = dt_j.rearrange("p (t c) -> p t c", t=T)
            gx_b = _bcast_outer(dv[:, :, 0], K)   # [128, K, T] step-0 over k
            gy_b = _bcast_outer(dv[:, :, 1], K)
            m_b = _bcast_outer(dv[:, :, 2], K)
            mlast = dt_j[:, 3 * T - 1:3 * T]      # [128,1] mask at t=T-1

            tr = tp.tile([128, K * T * C], F32)
            nc.sync.dma_start(out=tr, in_=trajs_r[i * 128:(i + 1) * 128, :])
            v = tr.rearrange("p (k t c) -> p k t c", k=K, t=T)
            lx, ly, sx, sy, rho = (v[:, :, :, q] for q in range(C))

            # ACT (all in the natural_log_exp table set)
            lsx = ip.tile([128, K, T], F32)
            nc.scalar.activation(lsx, sx, AF.Ln, scale=2.0)
            lsy = ip.tile([128, K, T], F32)
            nc.scalar.activation(lsy, sy, AF.Ln, scale=2.0)
            rsx = ip.tile([128, K, T], F32)
            nc.scalar.activation(rsx, lsx, AF.Exp, bias=bln2, scale=-1.0)
            rsy = ip.tile([128, K, T], F32)
            nc.scalar.activation(rsy, lsy, AF.Exp, bias=bln2, scale=-1.0)
            vab = ip.tile([128, K, 2, T], F32)
            nc.scalar.activation(vab[:, :, 0, :], rho, AF.Ln, bias=1.0, scale=1.0)
            nc.scalar.activation(vab[:, :, 1, :], rho, AF.Ln, bias=1.0, scale=-1.0)

            # DVE
            dx = ip.tile([128, K, T], F32)
            nc.vector.tensor_tensor(dx, gx_b, lx, OP.subtract)
            dy = ip.tile([128, K, T], F32)
            nc.vector.tensor_tensor(dy, gy_b, ly, OP.subtract)
            adx = ip.tile([128, K, T], F32)
            nc.scalar.activation(adx, dx, AF.Abs)
            ady = ip.tile([128, K, T], F32)
            nc.scalar.activation(ady, dy, AF.Abs)
            t1 = ip.tile([128, K, T], F32)
            nc.vector.tensor_tensor(t1, adx, rsx, OP.mult)
            t2 = ip.tile([128, K, T], F32)
            nc.vector.tensor_tensor(t2, ady, rsy, OP.mult)
            s = ip.tile([128, K, T], F32)
            nc.vector.tensor_tensor(s, lsx, lsy, OP.add)
            w1 = ip.tile([128, K, T], F32)
            nc.vector.tensor_tensor(w1, t1, t2, OP.add)
            w = ip.tile([128, K, T], F32)
            nc.vector.tensor_tensor(w, s, w1, OP.add)
            mw = ip.tile([128, K, T], F32)
            nc.vector.tensor_tensor(mw, w, m_b, OP.mult)
            dx2 = ip.tile([128, K, T], F32)
            nc.vector.tensor_tensor(dx2, dx, dx, OP.mult)
            dy2 = ip.tile([128, K, T], F32)
            nc.vector.tensor_tensor(dy2, dy, dy, OP.mult)
            d2 = ip.tile([128, K, T], F32)
            nc.vector.tensor_tensor(d2, dx2, dy2, OP.add)
            ld = ip.tile([128, K, T], F32)
            nc.scalar.activation(ld, d2, AF.Ln)
            dist = ip.tile([128, K, T], F32)
            nc.scalar.activation(dist, ld, AF.Exp, scale=0.5)
            mdist = ip.tile([128, K, T], F32)
            nc.vector.tensor_tensor(mdist, dist, m_b, OP.mult)

            # reductions over t -> packed [128, K] column slices
            nc.vector.tensor_reduce(R1[:, i, :], mw, AX.X, OP.add)
            nc.vector.tensor_reduce(R2[:, i, :], s, AX.X, OP.add)
            nc.vector.tensor_reduce(R3[:, i, :], vab, AX.XY, OP.add)
            nc.vector.tensor_reduce(R4[:, i, :], mdist, AX.X, OP.add)
            nc.vector.tensor_scalar(FDE[:, i, :], dist[:, :, T - 1], mlast, None,
                                    OP.mult)

        # ================= stage B: per-(l,b,n) mode softmax etc =============
        vw = [128, TILES * K]  # flat views

        lp = rp.tile(vw, F32)
        nc.scalar.activation(lp, P_sb.rearrange("p a b -> p (a b)"), AF.Ln)
        g = rp.tile([128, TILES, K], F32)
        nc.vector.tensor_tensor(g.rearrange("p a b -> p (a b)"), lp,
                                R1.rearrange("p a b -> p (a b)"), OP.subtract)
        gmx = rp.tile([128, TILES], F32)
        nc.vector.tensor_reduce(gmx, g, AX.X, OP.max)
        gs = rp.tile([128, TILES, K], F32)
        nc.vector.tensor_tensor(gs, g, _bcast_inner(gmx, K), OP.subtract)
        e = rp.tile([128, TILES, K], F32)
        nc.scalar.activation(e.rearrange("p a b -> p (a b)"),
                             gs.rearrange("p a b -> p (a b)"), AF.Exp)
        se = rp.tile([128, TILES], F32)
        nc.vector.tensor_reduce(se, e, AX.X, OP.add)
        ne = rp.tile([128, TILES, K], F32)
        nc.vector.tensor_tensor(ne.rearrange("p a b -> p (a b)"),
                                R1.rearrange("p a b -> p (a b)"),
                                e.rearrange("p a b -> p (a b)"), OP.mult)
        nes = rp.tile([128, TILES], F32)
        nc.vector.tensor_reduce(nes, ne, AX.X, OP.add)
        rse = rp.tile([128, TILES], F32)
        nc.vector.reciprocal(rse, se)
        nllpost = rp.tile([128, TILES], F32)
        nc.vector.tensor_tensor(nllpost, nes, rse, OP.mult)
        lnse = rp.tile([128, TILES], F32)
        nc.scalar.activation(lnse, se, AF.Ln)
        lse = rp.tile([128, TILES], F32)
        nc.vector.tensor_tensor(lse, lnse, gmx, OP.add)

        ent = rp.tile([128, TILES, K], F32)
        nc.vector.scalar_tensor_tensor(ent.rearrange("p a b -> p (a b)"),
                                       R3.rearrange("p a b -> p (a b)"), 0.5,
                                       R2.rearrange("p a b -> p (a b)"),
                                       OP.mult, OP.add)
        entmax = rp.tile([128, TILES], F32)
        nc.vector.tensor_reduce(entmax, ent, AX.X, OP.max)

        afk = rp.tile([128, TILES, K], F32)
        nc.vector.scalar_tensor_tensor(afk.rearrange("p a b -> p (a b)"),
                                       R4.rearrange("p a b -> p (a b)"), 1.0 / T,
                                       FDE.rearrange("p a b -> p (a b)"),
                                       OP.mult, OP.add)
        mfa = rp.tile([128, TILES], F32)
        nc.vector.tensor_reduce(mfa, afk, AX.X, OP.min)
        made = rp.tile([128, TILES], F32)
        nc.vector.tensor_reduce(made, R4, AX.X, OP.min)

        q1 = rp.tile([128, TILES], F32)
        nc.vector.tensor_scalar(q1, nllpost, -19.0, None, OP.mult)
        q2 = rp.tile([128, TILES], F32)
        nc.vector.scalar_tensor_tensor(q2, lse, -20.0, q1, OP.mult, OP.add)
        q3 = rp.tile([128, TILES], F32)
        nc.vector.scalar_tensor_tensor(q3, entmax, 40.0, q2, OP.mult, OP.add)
        main = rp.tile([128, TILES], F32)
        nc.vector.scalar_tensor_tensor(main, mfa, 100.0, q3, OP.mult, OP.add)
        # + ENT_CONST per row happens at the very end (host side would also
        # work, but keep the device output exact): fold into rowsum below.

        # ================= stage C: scores loss over levels ==================
        msc = rp.tile([128, TILES], F32)
        nc.vector.tensor_reduce(msc, S_sb, AX.X, OP.max)
        madeS = rp.tile([128, TILES], F32)
        nc.vector.tensor_scalar(madeS, made, 1.0 / T, None, OP.mult)

        def stdnorm_softmax(x16, sign, nm):
            # x16 [128,16] cols i = l*4 + bb; view [128, bb, l], normalize over l
            xv = x16.rearrange("p (l bb) -> p bb l", l=L)
            ms = rp.tile([128, DTILES], F32, name=f"ms_{nm}")
            nc.vector.tensor_reduce(ms, xv, AX.X, OP.add)
            xc = rp.tile([128, DTILES, L], F32, name=f"xc_{nm}")
            nc.vector.scalar_tensor_tensor(xc, _bcast_inner(ms, L), -1.0 / L, xv,
                                           OP.mult, OP.add)
            xc2 = rp.tile([128, DTILES, L], F32, name=f"xc2_{nm}")
            nc.vector.tensor_tensor(xc2.rearrange("p a b -> p (a b)"),
                                    xc.rearrange("p a b -> p (a b)"),
                                    xc.rearrange("p a b -> p (a b)"), OP.mult)
            ss = rp.tile([128, DTILES], F32, name=f"ss_{nm}")
            nc.vector.tensor_reduce(ss, xc2, AX.X, OP.add)
            lss = rp.tile([128, DTILES], F32, name=f"lss_{nm}")
            nc.scalar.activation(lss, ss, AF.Ln, scale=1.0 / (L - 1))
            sd = rp.tile([128, DTILES], F32, name=f"sd_{nm}")
            nc.scalar.activation(sd, lss, AF.Exp, scale=0.5)
            sdp = rp.tile([128, DTILES], F32, name=f"sdp_{nm}")
            nc.vector.tensor_scalar(sdp, sd, 1e-5, None, OP.add)
            rsd = rp.tile([128, DTILES], F32, name=f"rsd_{nm}")
            nc.vector.reciprocal(rsd, sdp)
            z = rp.tile([128, DTILES, L], F32, name=f"z_{nm}")
            nc.vector.tensor_tensor(z, xc, _bcast_inner(rsd, L), OP.mult)
            zz = rp.tile([128, DTILES, L], F32, name=f"zz_{nm}")
            nc.vector.tensor_scalar(zz.rearrange("p a b -> p (a b)"),
                                    z.rearrange("p a b -> p (a b)"), sign, None,
                                    OP.mult)
            mx = rp.tile([128, DTILES], F32, name=f"mx_{nm}")
            nc.vector.tensor_reduce(mx, zz, AX.X, OP.max)
            zs = rp.tile([128, DTILES, L], F32, name=f"zs_{nm}")
            nc.vector.tensor_tensor(zs, zz, _bcast_inner(mx, L), OP.subtract)
            ee = rp.tile([128, DTILES, L], F32, name=f"ee_{nm}")
            nc.scalar.activation(ee.rearrange("p a b -> p (a b)"),
                                 zs.rearrange("p a b -> p (a b)"), AF.Exp)
            ses = rp.tile([128, DTILES], F32, name=f"ses_{nm}")
            nc.vector.tensor_reduce(ses, ee, AX.X, OP.add)
            rs = rp.tile([128, DTILES], F32, name=f"rs_{nm}")
            nc.vector.reciprocal(rs, ses)
            pr = rp.tile([128, DTILES, L], F32, name=f"pr_{nm}")
            nc.vector.tensor_tensor(pr, ee, _bcast_inner(rs, L), OP.mult)
            return pr

        bests = stdnorm_softmax(madeS, -1.0, "a")
        pred = stdnorm_softmax(msc, 1.0, "b")
        df = rp.tile([128, DTILES * L], F32)
        nc.vector.tensor_tensor(df, bests.rearrange("p a b -> p (a b)"),
                                pred.rearrange("p a b -> p (a b)"), OP.subtract)
        df2 = rp.tile([128, DTILES * L], F32)
        nc.vector.tensor_tensor(df2, df, df, OP.mult)
        scsum = rp.tile([128, 1], F32)
        nc.vector.tensor_reduce(scsum, df2, AX.X, OP.add)

        # ================= stage D: per-core partial =========================
        rowsum = rp.tile([128, 1], F32)
        nc.vector.tensor_reduce(rowsum, main, AX.X, OP.add)
        # rowsum covers TILES=16 rows/partition; each row gets +ENT_CONST
        rowsum2 = rp.tile([128, 1], F32)
        nc.vector.tensor_scalar(rowsum2, rowsum, float(TILES) * ENT_CONST, None,
                                OP.add)
        tot = rp.tile([128, 1], F32)
        nc.vector.tensor_tensor(tot, rowsum2, scsum, OP.add)
        nc.sync.dma_start(out=out_d[:], in_=tot)

    nc.finalize()
    return nc


_NC = None


def _get_nc():
    global _NC
    if _NC is None:
        _NC = build_kernel()
    return _NC


def kernel(**inputs) -> np.ndarray:
    nc = _get_nc()
    trajs = np.ascontiguousarray(inputs["trajs"], dtype=np.float32)
    data = np.ascontiguousarray(inputs["data"], dtype=np.float32)
    probs = np.ascontiguousarray(inputs["probs"], dtype=np.float32)
    scores = np.ascontiguousarray(inputs["scores"], dtype=np.float32)

    in_maps = []
    for c in range(NCORES):
        sl = slice(c * Bc, (c + 1) * Bc)
        in_maps.append({
            "trajs": np.ascontiguousarray(trajs[:, sl]),
            "data": np.ascontiguousarray(data[sl]),
            "probs": np.ascontiguousarray(probs[:, sl]),
            "scores": np.ascontiguousarray(scores[:, sl]),
        })

    res = run_bass_kernel_spmd(nc, in_maps, list(range(NCORES)))
    total = 0.0
    for c in range(NCORES):
        total += np.asarray(res.results[c]["out"], dtype=np.float64).sum()
    B_full = Bc * NCORES
    return np.float32(total / (B_full * L * N))

